# revision 16
# baseline (speedup 1.0000x reference)
"""Binarized 4-layer MLP (8192x784 -> 6144 -> 6144 -> 6144 -> 10, log_softmax)
on 8 Trainium2 NeuronCores, data-parallel over the batch.

Per-core dataflow (batch slice of 1024, feature-major activations [feat, batch]):
  fc1: x @ sign(w1).T as a 2-term fp16 hi/lo split of x, with the two terms
       stacked along the contraction dim (1568 rows -> 13 k-tiles). fp16
       upconverts losslessly to the PE's e10m11 internal format and the
       weights are exactly +-1, so this reproduces fp32 accuracy.
  fc2/fc3: one level of Strassen over the 1024x6144 @ 6144x6144 binary
       matmul: 7 half-size products instead of 8 (12.5% fewer PE cycles).
       Activation-side combos are {-2,0,2} (exact in fp8, built on the DVE,
       pipelined behind the previous phase); weight-side combos are
       precomputed on the host. Products run in fp8e4 DoubleRow; all values
       are small integers so fp32 PSUM arithmetic stays bit-exact.
  fc4: fused into the fc3 f-loop, single fp16 pass (w4 and h3 in fp16).
  log_softmax: max-free (binarized logits are bounded, exp cannot overflow):
       out = l - ln(sum(exp(l))), with the per-column ln-sum carried in
       partition 32 of the logits tile through one PE transpose.
"""

import numpy as np
import ml_dtypes

import concourse.bass as bass
import concourse.mybir as mybir
from concourse import bacc
from concourse.tile import TileContext
from concourse.bass_utils import run_bass_kernel_spmd
from concourse.masks import make_identity

dt = mybir.dt

CORES = 8
B = 8192
BC = B // CORES          # 1024 batch rows per core
DIN = 784
KT1 = 13                 # fc1 contraction tiles: 2*784 = 1568 padded to 1664
K1P = KT1 * 128
DH = 6144
MT = DH // 128           # 48 feature tiles
KBS = 12                 # DoubleRow blocks per 3072 Strassen half
NF = 24                  # 128-wide output chunks per 3072 block-column
DOUT = 10
NH = BC // 512           # 2 moving halves of 512
MQ = 12                  # fc1 m-groups (w1 streamed per 4 m-tiles)
MPQ = MT // MQ

BF16 = ml_dtypes.bfloat16
FP8 = mybir.dt.np(dt.float8e4)

last_exec_time_ns = None

# Strassen product order: direct-operand chains (M4) first; combines fire as
# soon as their inputs exist. M3/M4 read h directly, the rest read S-combos.
ORDER = [4, 7, 5, 1, 2, 3, 6]


def _build_program():
    nc = bacc.Bacc("TRN2", target_bir_lowering=False, debug=False,
                   num_devices=CORES)

    xt = nc.dram_tensor("xt", [128, KT1, BC], dt.float16,
                        kind="ExternalInput").ap()
    w1t = nc.dram_tensor("w1t", [MQ, 128, KT1, MPQ * 128], dt.float16,
                         kind="ExternalInput").ap()
    w2s = nc.dram_tensor("w2s", [NF, 128, 7, KBS, 2, 128], dt.float8e4,
                         kind="ExternalInput").ap()
    w3s = nc.dram_tensor("w3s", [NF, 128, 7, KBS, 2, 128], dt.float8e4,
                         kind="ExternalInput").ap()
    w4p = nc.dram_tensor("w4p", [128, MT, DOUT], dt.float16,
                         kind="ExternalInput").ap()
    b1p = nc.dram_tensor("b1p", [128, MT], dt.float32, kind="ExternalInput").ap()
    b2p = nc.dram_tensor("b2p", [128, MT], dt.float32, kind="ExternalInput").ap()
    b3p = nc.dram_tensor("b3p", [128, MT], dt.float32, kind="ExternalInput").ap()
    b4p = nc.dram_tensor("b4p", [DOUT, 1], dt.float32, kind="ExternalInput").ap()
    out = nc.dram_tensor("out", [BC, DOUT], dt.float32, kind="ExternalOutput").ap()

    DR = mybir.MatmulPerfMode.DoubleRow
    AF = mybir.ActivationFunctionType
    ADD = mybir.AluOpType.add
    SUB = mybir.AluOpType.subtract
    MULT = mybir.AluOpType.mult

    def tt(out_, a, b_, op):
        nc.vector.scalar_tensor_tensor(out_, a, 1.0, b_, MULT, op)

    with TileContext(nc) as tc:
        with tc.tile_pool(name="consts", bufs=1) as cpool, \
             tc.tile_pool(name="h2p", bufs=1) as h2pool:
            with tc.tile_pool(name="h1p", bufs=1) as h1pool, \
                 tc.tile_pool(name="s2a", bufs=1) as s2apool:
                with tc.tile_pool(name="xtq", bufs=1) as xtq, \
                     tc.tile_pool(name="w1pool", bufs=2) as w1pool, \
                     tc.tile_pool(name="ps1", bufs=3, space="PSUM") as ps1:
                    # --- startup DMAs in fc1 consumption order
                    # (k-interleaved), alternating dispatch engines ---
                    w1q0 = {}
                    xt_half = {}
                    for k in range(KT1):
                        if k == 0:
                            for n in range(NH):
                                tx = xtq.tile([128, 512], dt.float16,
                                              tag=f"xt0_{n}")
                                nc.sync.dma_start(
                                    out=tx[:],
                                    in_=xt[:, 0, n * 512:(n + 1) * 512])
                                xt_half[(0, n)] = tx[:, :]
                        else:
                            tx = xtq.tile([128, BC], dt.float16, tag=f"xt_{k}")
                            nc.sync.dma_start(out=tx[:], in_=xt[:, k, :])
                            for n in range(NH):
                                xt_half[(k, n)] = tx[:, n * 512:(n + 1) * 512]
                        tw = xtq.tile([128, MPQ * 128], dt.float16,
                                      tag=f"w1q0_{k}")
                        nc.gpsimd.dma_start(out=tw[:], in_=w1t[0, :, k, :])
                        w1q0[k] = tw
                    # first weight tiles of fc2/fc3 (avoid waiting on the
                    # SBUF zone recycle at the phase boundary)
                    w2pre = {}
                    w3pre = {}
                    for i, eng in ((4, nc.sync), (7, nc.gpsimd)):
                        t2 = cpool.tile([128, KBS, 2, 128], dt.float8e4,
                                        tag=f"w2pre{i}", name=f"w2pre{i}")
                        eng.dma_start(out=t2[:], in_=w2s[0, :, i - 1])
                        w2pre[i] = t2
                        t3 = cpool.tile([128, KBS, 2, 128], dt.float8e4,
                                        tag=f"w3pre{i}", name=f"w3pre{i}")
                        eng.dma_start(out=t3[:], in_=w3s[0, :, i - 1])
                        w3pre[i] = t3

                    b1_sb = cpool.tile([128, MT], dt.float32)
                    nc.sync.dma_start(out=b1_sb[:], in_=b1p[:])
                    b2_sb = cpool.tile([128, MT], dt.float32)
                    nc.gpsimd.dma_start(out=b2_sb[:], in_=b2p[:])
                    b3_sb = cpool.tile([128, MT], dt.float32)
                    nc.sync.dma_start(out=b3_sb[:], in_=b3p[:])
                    b4_sb = cpool.tile([DOUT, 1], dt.float32)
                    nc.gpsimd.dma_start(out=b4_sb[:], in_=b4p[:])
                    w4_sb = cpool.tile([128, MT, DOUT], dt.float16)
                    nc.sync.dma_start(out=w4_sb[:], in_=w4p[:])
                    ident = cpool.tile([33, 33], dt.float32)
                    make_identity(nc, ident[:])
                    ones_sb = cpool.tile([DOUT, 1], dt.float16)
                    nc.vector.memset(ones_sb[:], 1.0)
                    # pre-warm Exp/Ln activation tables
                    warm = cpool.tile([1, 1], dt.float32)
                    nc.scalar.activation(warm[:], ident[0:1, 0:1], AF.Exp)
                    nc.scalar.activation(warm[:], warm[:], AF.Ln)

                    h1 = h1pool.tile([128, MT, BC], dt.float8e4)
                    h2 = h2pool.tile([128, MT, BC], dt.float8e4)

                    # px=0 Strassen S-tiles for fc2, chunk-built inside the
                    # fc1 m-loop as soon as both operand kt rows exist
                    S2A = {}
                    for i in (1, 2, 5, 6, 7):
                        S2A[i] = s2apool.tile([128, 24, 256], dt.float8e4,
                                              tag=f"s2a{i}", name=f"s2a{i}")

                    def s2_chunks(kk, c0):
                        lo1, lo2 = c0, c0 + 256
                        hi1, hi2 = 512 + c0, 512 + c0 + 256
                        a11 = h1[:, kk, lo1:lo2]
                        a12 = h1[:, 24 + kk, lo1:lo2]
                        a21 = h1[:, kk, hi1:hi2]
                        a22 = h1[:, 24 + kk, hi1:hi2]
                        tt(S2A[1][:, kk, :], a11, a22, ADD)
                        tt(S2A[2][:, kk, :], a21, a22, ADD)
                        tt(S2A[5][:, kk, :], a11, a12, ADD)
                        tt(S2A[6][:, kk, :], a21, a11, SUB)
                        tt(S2A[7][:, kk, :], a12, a22, SUB)

                    # ---------------- fc1 ----------------
                    for q in range(MQ):
                        if q == 0:
                            def lhs1(k, mi):
                                return w1q0[k][:, mi * 128:(mi + 1) * 128]
                        else:
                            w1q = w1pool.tile([128, KT1, MPQ * 128],
                                              dt.float16, tag="w1")
                            nc.sync.dma_start(out=w1q[:], in_=w1t[q])

                            def lhs1(k, mi, w1q=w1q):
                                return w1q[:, k, mi * 128:(mi + 1) * 128]
                        for mi in range(MPQ):
                            m = q * MPQ + mi
                            psum = ps1.tile([128, BC], dt.float32, tag="ps1")
                            for k in range(KT1):
                                for n in range(NH):
                                    nc.tensor.matmul(
                                        psum[:, n * 512:(n + 1) * 512],
                                        lhs1(k, mi),
                                        xt_half[(k, n)],
                                        start=(k == 0),
                                        stop=(k == KT1 - 1),
                                    )
                            nc.scalar.sign(h1[:, m, :], psum[:, :],
                                           bias=b1_sb[:, m:m + 1])
                            if m >= 24:
                                s2_chunks(m - 24, 0)

                # ------------- fc2 (one-level Strassen) -------------
                with tc.tile_pool(name="s2b", bufs=1) as s2bpool, \
                     tc.tile_pool(name="w2pool", bufs=6) as w2pool, \
                     tc.tile_pool(name="c2pool", bufs=1) as cp2, \
                     tc.tile_pool(name="ps2", bufs=1, space="PSUM") as ps2:
                    dmae = [nc.sync, nc.gpsimd]
                    nd = [0]

                    # px=1 S-tiles: whole-tile builds on the (idle) DVE while
                    # the px=0 groups run on the PE
                    c1 = 256
                    A11b = h1[:, 0:24, c1:c1 + 256]
                    A12b = h1[:, 24:48, c1:c1 + 256]
                    A21b = h1[:, 0:24, 512 + c1:512 + c1 + 256]
                    A22b = h1[:, 24:48, 512 + c1:512 + c1 + 256]
                    S2B = {}
                    for i, a, b_, op in ((1, A11b, A22b, ADD),
                                         (2, A21b, A22b, ADD),
                                         (5, A11b, A12b, ADD),
                                         (6, A21b, A11b, SUB),
                                         (7, A12b, A22b, SUB)):
                        t = s2bpool.tile([128, 24, 256], dt.float8e4,
                                         tag=f"s2b{i}", name=f"s2b{i}")
                        tt(t[:], a, b_, op)
                        S2B[i] = t

                    # fc2 loops: px outer, f inner
                    for px in range(2):
                        c0 = px * 256
                        S = S2A if px == 0 else S2B

                        def moving2(i, blk, c0=c0, S=S):
                            if i == 3:
                                return h1[:, 2 * blk:2 * blk + 2,
                                          c0:c0 + 256]
                            if i == 4:
                                return h1[:, 24 + 2 * blk:24 + 2 * blk + 2,
                                          512 + c0:512 + c0 + 256]
                            return S[i][:, 2 * blk:2 * blk + 2, :]

                        for f in range(NF):
                            wts = {}
                            for i in ORDER:
                                if px == 0 and f == 0 and i in w2pre:
                                    wts[i] = w2pre[i]
                                else:
                                    t = w2pool.tile([128, KBS, 2, 128],
                                                    dt.float8e4, tag="w2")
                                    eng = dmae[nd[0] % 2]
                                    nd[0] += 1
                                    eng.dma_start(out=t[:],
                                                  in_=w2s[f, :, i - 1])
                                    wts[i] = t
                            # 7 quarter-size products packed
                            # pairwise into 2KB PSUM banks; pairs chosen by
                            # lifetime so bufs=1 tags never stall the PE
                            pA = ps2.tile([128, 512], dt.float32, tag="pA",
                                          name="pA")
                            pB = ps2.tile([128, 512], dt.float32, tag="pB",
                                          name="pB")
                            pC = ps2.tile([128, 512], dt.float32, tag="pC",
                                          name="pC")
                            pD = ps2.tile([128, 256], dt.float32, tag="pD",
                                          name="pD")
                            psm = {4: pA[:, 0:256], 7: pA[:, 256:512],
                                   5: pB[:, 0:256], 1: pB[:, 256:512],
                                   2: pC[:, 0:256], 3: pC[:, 256:512],
                                   6: pD[:, 0:256]}

                            def ctile(tag, bufs=1):
                                return cp2.tile([128, 256], dt.float32,
                                                tag=tag, name=tag, bufs=bufs)
                            for i in ORDER:
                                for blk in range(KBS):
                                    nc.tensor.matmul(
                                        psm[i], wts[i][:, blk],
                                        moving2(i, blk),
                                        start=(blk == 0),
                                        stop=(blk == KBS - 1),
                                        perf_mode=DR,
                                    )
                                if i == 4:
                                    m4s = ctile("m4s")
                                    nc.scalar.activation(m4s[:], psm[4],
                                                         AF.Identity)
                                elif i == 7:
                                    x1 = ctile("x1")
                                    tt(x1[:], m4s[:], psm[7], ADD)
                                elif i == 5:
                                    m5s = ctile("m5s")
                                    nc.scalar.activation(m5s[:], psm[5],
                                                         AF.Identity)
                                elif i == 1:
                                    x2 = ctile("x2")
                                    tt(x2[:], x1[:], psm[1], ADD)
                                    c11 = ctile("c11", 2)
                                    tt(c11[:], x2[:], m5s[:], SUB)
                                    nc.scalar.sign(h2[:, f, c0:c0 + 256],
                                                   c11[:],
                                                   bias=b2_sb[:, f:f + 1])
                                elif i == 2:
                                    c21 = ctile("c21", 2)
                                    tt(c21[:], m4s[:], psm[2], ADD)
                                    nc.scalar.sign(
                                        h2[:, f, 512 + c0:512 + c0 + 256],
                                        c21[:], bias=b2_sb[:, f:f + 1])
                                    m2s = ctile("m2s")
                                    nc.scalar.activation(m2s[:], psm[2],
                                                         AF.Identity)
                                elif i == 3:
                                    c12 = ctile("c12", 2)
                                    tt(c12[:], m5s[:], psm[3], ADD)
                                    nc.scalar.sign(
                                        h2[:, 24 + f, c0:c0 + 256],
                                        c12[:],
                                        bias=b2_sb[:, 24 + f:25 + f])
                                    y2 = ctile("y2")
                                    nc.vector.scalar_tensor_tensor(
                                        y2[:], m2s[:], -1.0, psm[1],
                                        MULT, ADD)
                                    y3 = ctile("y3")
                                    tt(y3[:], y2[:], psm[3], ADD)
                                elif i == 6:
                                    c22 = ctile("c22", 2)
                                    tt(c22[:], y3[:], psm[6], ADD)
                                    nc.scalar.sign(
                                        h2[:, 24 + f,
                                           512 + c0:512 + c0 + 256],
                                        c22[:],
                                        bias=b2_sb[:, 24 + f:25 + f])

            # ------------- fc3 (Strassen) + fused fc4 -------------
            with tc.tile_pool(name="lgp", bufs=1, space="PSUM") as lgp, \
                 tc.tile_pool(name="lgsbp", bufs=1) as lgsbp:
                lg_psum = lgp.tile([DOUT, BC], dt.float32)
                with tc.tile_pool(name="s3pool", bufs=1) as s3pool, \
                     tc.tile_pool(name="w3pool", bufs=6) as w3pool, \
                     tc.tile_pool(name="c3pool", bufs=1) as cp3, \
                     tc.tile_pool(name="h3pool", bufs=8) as h3pool, \
                     tc.tile_pool(name="ps3", bufs=1, space="PSUM") as ps3:
                    dmae3 = [nc.sync, nc.gpsimd]
                    nd3 = [0]
                    # all 10 S-tiles (both px quarters) up front
                    S3 = {}
                    for px in range(2):
                        c0 = px * 256
                        A11 = h2[:, 0:24, c0:c0 + 256]
                        A12 = h2[:, 24:48, c0:c0 + 256]
                        A21 = h2[:, 0:24, 512 + c0:512 + c0 + 256]
                        A22 = h2[:, 24:48, 512 + c0:512 + c0 + 256]
                        for i, a, b_, op in ((7, A12, A22, SUB),
                                             (5, A11, A12, ADD),
                                             (1, A11, A22, ADD),
                                             (2, A21, A22, ADD),
                                             (6, A21, A11, SUB)):
                            t = s3pool.tile([128, 24, 256], dt.float8e4,
                                            tag=f"s3_{i}_{px}",
                                            name=f"s3_{i}_{px}")
                            tt(t[:], a, b_, op)
                            S3[(i, px)] = t

                    h3_tiles = {}

                    def fc4_mms(m):
                        t_h3 = h3_tiles[m]
                        for n in range(NH):
                            nc.tensor.matmul(
                                lg_psum[:, n * 512:(n + 1) * 512],
                                w4_sb[:, m, :],
                                t_h3[:, n * 512:(n + 1) * 512],
                                start=(m == 0),
                                stop=(m == MT - 1),
                            )

                    def h3_store(m, cols, csb):
                        sl = h3_tiles[m][:, cols[0]:cols[1]]
                        nc.scalar.activation(sl, csb[:], AF.Identity,
                                             bias=b3_sb[:, m:m + 1])
                        nc.vector.tensor_scalar(sl, sl, 1.0, -1.0,
                                                mybir.AluOpType.min,
                                                mybir.AluOpType.max)

                    # fc3 loops: f outer (so h3/fc4 drain promptly), px inner
                    for f in range(NF):
                        for m in (f, 24 + f):
                            h3_tiles[m] = h3pool.tile([128, BC], dt.float16,
                                                      tag="h3", name="h3")
                        for px in range(2):
                            c0 = px * 256

                            def moving3(i, blk, c0=c0, px=px):
                                if i == 3:
                                    return h2[:, 2 * blk:2 * blk + 2,
                                              c0:c0 + 256]
                                if i == 4:
                                    return h2[:,
                                              24 + 2 * blk:24 + 2 * blk + 2,
                                              512 + c0:512 + c0 + 256]
                                return S3[(i, px)][:, 2 * blk:2 * blk + 2, :]

                            wts = {}
                            for i in ORDER:
                                if px == 0 and f == 0 and i in w3pre:
                                    wts[i] = w3pre[i]
                                else:
                                    t = w3pool.tile([128, KBS, 2, 128],
                                                    dt.float8e4, tag="w3")
                                    eng = dmae3[nd3[0] % 2]
                                    nd3[0] += 1
                                    eng.dma_start(out=t[:],
                                                  in_=w3s[f, :, i - 1])
                                    wts[i] = t
                            qA = ps3.tile([128, 512], dt.float32,
                                          tag="qA", name="qA")
                            qB = ps3.tile([128, 512], dt.float32,
                                          tag="qB", name="qB")
                            qC = ps3.tile([128, 512], dt.float32,
                                          tag="qC", name="qC")
                            qD = ps3.tile([128, 256], dt.float32,
                                          tag="qD", name="qD")
                            psm = {4: qA[:, 0:256], 7: qA[:, 256:512],
                                   5: qB[:, 0:256], 1: qB[:, 256:512],
                                   2: qC[:, 0:256], 3: qC[:, 256:512],
                                   6: qD[:, 0:256]}

                            def ctile3(tag, bufs=1):
                                return cp3.tile([128, 256], dt.float32,
                                                tag=tag, name=tag, bufs=bufs)
                            for i in ORDER:
                                for blk in range(KBS):
                                    nc.tensor.matmul(
                                        psm[i], wts[i][:, blk],
                                        moving3(i, blk),
                                        start=(blk == 0),
                                        stop=(blk == KBS - 1),
                                        perf_mode=DR,
                                    )
                                if i == 4:
                                    m4s = ctile3("f3m4s")
                                    nc.scalar.activation(m4s[:], psm[4],
                                                         AF.Identity)
                                elif i == 7:
                                    x1 = ctile3("f3x1")
                                    tt(x1[:], m4s[:], psm[7], ADD)
                                elif i == 5:
                                    m5s = ctile3("f3m5s")
                                    nc.scalar.activation(m5s[:], psm[5],
                                                         AF.Identity)
                                elif i == 1:
                                    x2 = ctile3("f3x2")
                                    tt(x2[:], x1[:], psm[1], ADD)
                                    c11 = ctile3("f3c11", 2)
                                    tt(c11[:], x2[:], m5s[:], SUB)
                                    h3_store(f, (c0, c0 + 256), c11)
                                elif i == 2:
                                    c21 = ctile3("f3c21", 2)
                                    tt(c21[:], m4s[:], psm[2], ADD)
                                    h3_store(f, (512 + c0, 512 + c0 + 256),
                                             c21)
                                    m2s = ctile3("f3m2s")
                                    nc.scalar.activation(m2s[:], psm[2],
                                                         AF.Identity)
                                elif i == 3:
                                    c12 = ctile3("f3c12", 2)
                                    tt(c12[:], m5s[:], psm[3], ADD)
                                    h3_store(24 + f, (c0, c0 + 256), c12)
                                    y2 = ctile3("f3y2")
                                    nc.vector.scalar_tensor_tensor(
                                        y2[:], m2s[:], -1.0, psm[1],
                                        MULT, ADD)
                                    y3 = ctile3("f3y3")
                                    tt(y3[:], y2[:], psm[3], ADD)
                                elif i == 6:
                                    c22 = ctile3("f3c22", 2)
                                    tt(c22[:], y3[:], psm[6], ADD)
                                    h3_store(24 + f,
                                             (512 + c0, 512 + c0 + 256), c22)
                        # fc4, pipelined one f behind
                        if f > 0:
                            fc4_mms(f - 1)
                            fc4_mms(24 + f - 1)
                    fc4_mms(NF - 1)
                    fc4_mms(24 + NF - 1)

                # ------------- bias + log_softmax (max-free) -------------
                # logits are bounded (|l| < 40), so exp() cannot overflow
                # fp32 and the rowmax subtraction is unnecessary:
                # out = l - ln(sum(exp(l))). Partition 32 of lg_sb holds the
                # per-column ln-sum so one PE transpose carries both.
                lg_sb = lgsbp.tile([33, BC], dt.float32)
                nc.scalar.activation(lg_sb[0:DOUT, :], lg_psum[:],
                                     AF.Identity, bias=b4_sb[:, 0:1])
                NJ = BC // 128
                with tc.tile_pool(name="tp", bufs=1, space="PSUM") as tpp, \
                     tc.tile_pool(name="sm", bufs=1) as smp:
                    ex_sb = smp.tile([DOUT, BC], dt.float16, tag="ex")
                    nc.scalar.activation(ex_sb[:], lg_psum[:], AF.Exp,
                                         bias=b4_sb[:, 0:1])
                    sums_ps = tpp.tile([1, BC], dt.float32, tag="sums")
                    for n in range(NH):
                        nc.tensor.matmul(
                            sums_ps[:, n * 512:(n + 1) * 512],
                            ones_sb[:, 0:1],
                            ex_sb[:, n * 512:(n + 1) * 512],
                        )
                    nc.scalar.activation(lg_sb[32:33, :], sums_ps[:], AF.Ln)
                    for j in range(NJ):
                        tp = tpp.tile([128, 33], dt.float32, tag=f"tp{j%4}")
                        nc.tensor.transpose(
                            tp[:], lg_sb[:, j * 128:(j + 1) * 128], ident[:])
                        res = smp.tile([128, DOUT], dt.float32,
                                       tag=f"res{j}")
                        nc.vector.tensor_scalar(res[:], tp[:, 0:DOUT],
                                                tp[:, 32:33], None,
                                                mybir.AluOpType.subtract)
                        nc.sync.dma_start(
                            out=out[j * 128:(j + 1) * 128, :], in_=res[:])

    nc.compile()
    return nc


def _pack_inputs(x, w1, b1, w2, b2, w3, b3, w4, b4):
    """Host-side packing into the device layouts. Shared tensors are packed
    once; only xt differs per core."""
    f32 = np.float32
    f16 = np.float16
    x = np.asarray(x, f32).reshape(B, DIN)

    # fc1 weights: sign(w1).T stacked twice (hi/lo terms share the weights),
    # padded to [1664, 6144], layout [q, p, k, m]
    s1 = np.sign(np.asarray(w1, f32))                       # [DH, DIN]
    s1t = np.zeros((K1P, DH), f16)
    s1t[:DIN] = s1.T
    s1t[DIN:2 * DIN] = s1.T
    w1t = np.ascontiguousarray(
        s1t.reshape(KT1, 128, MQ, MPQ * 128).transpose(2, 1, 0, 3))

    # fc2/fc3 weights: Strassen T-combos of sign(w).T, DoubleRow layout per
    # 128-wide output chunk: [fo, p, 7, blk, i2, f']
    def pack_strassen(w):
        sm = np.sign(np.asarray(w, f32)).T                  # [in, out] = B
        H = DH // 2
        B11 = sm[:H, :H]
        B12 = sm[:H, H:]
        B21 = sm[H:, :H]
        B22 = sm[H:, H:]
        Ts = [B11 + B22, B11, B12 - B22, B21 - B11, B22, B11 + B12,
              B21 + B22]

        def pack_t(t):   # [3072, 3072] -> [fo, p, blk, i2, f']
            r = t.reshape(KBS, 2, 128, NF, 128)
            return r.transpose(3, 2, 0, 1, 4)

        return np.ascontiguousarray(
            np.stack([pack_t(t) for t in Ts], axis=2)).astype(FP8)

    w2sp = pack_strassen(w2)
    w3sp = pack_strassen(w3)

    # fc4 weights: w4.T in fp16, layout [p, j, c]
    w4t = np.asarray(w4, f32).T.astype(f16)                 # [DH, DOUT]
    w4pk = np.ascontiguousarray(w4t.reshape(MT, 128, DOUT).transpose(1, 0, 2))

    def pack_b(b):
        return np.ascontiguousarray(np.asarray(b, f32).reshape(MT, 128).T)

    b1pk, b2pk, b3pk = pack_b(b1), pack_b(b2), pack_b(b3)
    b4pk = np.asarray(b4, f32).reshape(DOUT, 1)

    shared = {"w1t": w1t, "w2s": w2sp, "w3s": w3sp, "w4p": w4pk,
              "b1p": b1pk, "b2p": b2pk, "b3p": b3pk, "b4p": b4pk}

    # per-core x: fp16 hi/lo split stacked along contraction, layout [p, k, n]
    in_maps = []
    for c in range(CORES):
        xc = x[c * BC:(c + 1) * BC]                         # [BC, DIN]
        hi = xc.astype(f16)
        lo = (xc - hi.astype(f32)).astype(f16)
        arr = np.zeros((K1P, BC), f16)
        arr[:DIN] = hi.T
        arr[DIN:2 * DIN] = lo.T
        xtc = np.ascontiguousarray(arr.reshape(KT1, 128, BC).transpose(1, 0, 2))
        in_maps.append({"xt": xtc, **shared})
    return in_maps


_cached_nc = None


def kernel(x, w1, b1, w2, b2, w3, b3, w4, b4):
    global _cached_nc, last_exec_time_ns
    import os
    trace = bool(int(os.environ.get("KERNEL_TRACE", "0")))
    if _cached_nc is None:
        _cached_nc = _build_program()
    in_maps = _pack_inputs(x, w1, b1, w2, b2, w3, b3, w4, b4)
    res = run_bass_kernel_spmd(_cached_nc, in_maps, list(range(CORES)),
                               trace=trace)
    last_exec_time_ns = res.exec_time_ns
    return np.concatenate([res.results[c]["out"] for c in range(CORES)], axis=0)


# revision 18
# speedup vs baseline: 1.0805x; 1.0805x over previous
"""Binarized 4-layer MLP (8192x784 -> 6144 -> 6144 -> 6144 -> 10, log_softmax)
on 8 Trainium2 NeuronCores, data-parallel over the batch.

Per-core dataflow (batch slice of 1024, feature-major activations [feat, batch]):
  fc1: x @ sign(w1).T as a 2-term fp16 hi/lo split of x, with the two terms
       stacked along the contraction dim (1568 rows -> 13 k-tiles). fp16
       upconverts losslessly to the PE's e10m11 internal format and the
       weights are exactly +-1, so this reproduces fp32 accuracy.
  fc2/fc3: one level of Strassen over the 1024x6144 @ 6144x6144 binary
       matmul: 7 half-size products instead of 8 (12.5% fewer PE cycles).
       Activation-side combos are {-2,0,2} (exact in fp8, built on the DVE,
       pipelined behind the previous phase); weight-side combos are
       precomputed on the host. Products run in fp8e4 DoubleRow; all values
       are small integers so fp32 PSUM arithmetic stays bit-exact.
  fc4: fused into the fc3 f-loop, single fp16 pass (w4 and h3 in fp16).
  log_softmax: max-free (binarized logits are bounded, exp cannot overflow):
       out = l - ln(sum(exp(l))), with the per-column ln-sum carried in
       partition 32 of the logits tile through one PE transpose.
"""

import numpy as np
import ml_dtypes

import concourse.bass as bass
import concourse.mybir as mybir
from concourse import bacc
from concourse.tile import TileContext
from concourse.bass_utils import run_bass_kernel_spmd
from concourse.masks import make_identity

dt = mybir.dt

CORES = 8
B = 8192
BC = B // CORES          # 1024 batch rows per core
DIN = 784
KT1 = 13                 # fc1 contraction tiles: 2*784 = 1568 padded to 1664
K1P = KT1 * 128
DH = 6144
MT = DH // 128           # 48 feature tiles
KBS = 12                 # DoubleRow blocks per 3072 Strassen half
NF = 24                  # 128-wide output chunks per 3072 block-column
DOUT = 10
NH = BC // 512           # 2 moving halves of 512
MQ = 12                  # fc1 m-groups (w1 streamed per 4 m-tiles)
MPQ = MT // MQ

BF16 = ml_dtypes.bfloat16
FP8 = mybir.dt.np(dt.float8e4)

last_exec_time_ns = None

# Strassen product order: direct-operand chains (M4) first; combines fire as
# soon as their inputs exist. M3/M4 read h directly, the rest read S-combos.
ORDER = [4, 7, 5, 1, 2, 3, 6]


def _build_program():
    nc = bacc.Bacc("TRN2", target_bir_lowering=False, debug=False,
                   num_devices=CORES)

    xt = nc.dram_tensor("xt", [128, KT1, BC], dt.float16,
                        kind="ExternalInput").ap()
    w1t = nc.dram_tensor("w1t", [MQ, 128, KT1, MPQ * 128], dt.float16,
                         kind="ExternalInput").ap()
    w2s = nc.dram_tensor("w2s", [NF, 128, 7, KBS, 2, 128], dt.float8e4,
                         kind="ExternalInput").ap()
    w3s = nc.dram_tensor("w3s", [NF, 128, 7, KBS, 2, 128], dt.float8e4,
                         kind="ExternalInput").ap()
    w4p = nc.dram_tensor("w4p", [128, MT, DOUT], dt.float16,
                         kind="ExternalInput").ap()
    b1p = nc.dram_tensor("b1p", [128, MT], dt.float32, kind="ExternalInput").ap()
    b2p = nc.dram_tensor("b2p", [128, MT], dt.float32, kind="ExternalInput").ap()
    b3p = nc.dram_tensor("b3p", [128, MT], dt.float32, kind="ExternalInput").ap()
    b4p = nc.dram_tensor("b4p", [DOUT, 1], dt.float32, kind="ExternalInput").ap()
    out = nc.dram_tensor("out", [BC, DOUT], dt.float32, kind="ExternalOutput").ap()

    DR = mybir.MatmulPerfMode.DoubleRow
    AF = mybir.ActivationFunctionType
    ADD = mybir.AluOpType.add
    SUB = mybir.AluOpType.subtract
    MULT = mybir.AluOpType.mult

    def tt(out_, a, b_, op):
        nc.vector.scalar_tensor_tensor(out_, a, 1.0, b_, MULT, op)

    with TileContext(nc) as tc:
        with tc.tile_pool(name="consts", bufs=1) as cpool, \
             tc.tile_pool(name="h2p", bufs=1) as h2pool:
            with tc.tile_pool(name="h1p", bufs=1) as h1pool, \
                 tc.tile_pool(name="s2a", bufs=1) as s2apool:
                with tc.tile_pool(name="xtq", bufs=1) as xtq, \
                     tc.tile_pool(name="w1pool", bufs=2) as w1pool, \
                     tc.tile_pool(name="ps1", bufs=3, space="PSUM") as ps1:
                    # --- startup DMAs in fc1 consumption order
                    # (k-interleaved), alternating dispatch engines ---
                    w1q0 = {}
                    xt_half = {}
                    for k in range(KT1):
                        if k == 0:
                            for n in range(NH):
                                tx = xtq.tile([128, 512], dt.float16,
                                              tag=f"xt0_{n}")
                                nc.sync.dma_start(
                                    out=tx[:],
                                    in_=xt[:, 0, n * 512:(n + 1) * 512])
                                xt_half[(0, n)] = tx[:, :]
                        else:
                            tx = xtq.tile([128, BC], dt.float16, tag=f"xt_{k}")
                            nc.sync.dma_start(out=tx[:], in_=xt[:, k, :])
                            for n in range(NH):
                                xt_half[(k, n)] = tx[:, n * 512:(n + 1) * 512]
                        tw = xtq.tile([128, MPQ * 128], dt.float16,
                                      tag=f"w1q0_{k}")
                        nc.gpsimd.dma_start(out=tw[:], in_=w1t[0, :, k, :])
                        w1q0[k] = tw
                    # first weight tiles of fc2/fc3 (avoid waiting on the
                    # SBUF zone recycle at the phase boundary)
                    w2pre = {}
                    w3pre = {}
                    for i, eng in ((4, nc.sync), (7, nc.gpsimd)):
                        t2 = cpool.tile([128, KBS, 2, 128], dt.float8e4,
                                        tag=f"w2pre{i}", name=f"w2pre{i}")
                        eng.dma_start(out=t2[:], in_=w2s[0, :, i - 1])
                        w2pre[i] = t2
                        t3 = cpool.tile([128, KBS, 2, 128], dt.float8e4,
                                        tag=f"w3pre{i}", name=f"w3pre{i}")
                        eng.dma_start(out=t3[:], in_=w3s[0, :, i - 1])
                        w3pre[i] = t3

                    b1_sb = cpool.tile([128, MT], dt.float32)
                    nc.sync.dma_start(out=b1_sb[:], in_=b1p[:])
                    b2_sb = cpool.tile([128, MT], dt.float32)
                    nc.gpsimd.dma_start(out=b2_sb[:], in_=b2p[:])
                    b3_sb = cpool.tile([128, MT], dt.float32)
                    nc.sync.dma_start(out=b3_sb[:], in_=b3p[:])
                    b4_sb = cpool.tile([DOUT, 1], dt.float32)
                    nc.gpsimd.dma_start(out=b4_sb[:], in_=b4p[:])
                    w4_sb = cpool.tile([128, MT, DOUT], dt.float16)
                    nc.sync.dma_start(out=w4_sb[:], in_=w4p[:])
                    ident = cpool.tile([33, 33], dt.float32)
                    make_identity(nc, ident[:])
                    ones_sb = cpool.tile([DOUT, 1], dt.float16)
                    nc.vector.memset(ones_sb[:], 1.0)
                    # pre-warm Exp/Ln activation tables
                    warm = cpool.tile([1, 1], dt.float32)
                    nc.scalar.activation(warm[:], ident[0:1, 0:1], AF.Exp)
                    nc.scalar.activation(warm[:], warm[:], AF.Ln)

                    h1 = h1pool.tile([128, MT, BC], dt.float8e4)
                    h2 = h2pool.tile([128, MT, BC], dt.float8e4)

                    # px=0 Strassen S-tiles for fc2, chunk-built inside the
                    # fc1 m-loop as soon as both operand kt rows exist
                    S2A = {}
                    for i in (1, 2, 5, 6, 7):
                        S2A[i] = s2apool.tile([128, 24, 256], dt.float8e4,
                                              tag=f"s2a{i}", name=f"s2a{i}")

                    def s2_chunks(kk, c0):
                        lo1, lo2 = c0, c0 + 256
                        hi1, hi2 = 512 + c0, 512 + c0 + 256
                        a11 = h1[:, kk, lo1:lo2]
                        a12 = h1[:, 24 + kk, lo1:lo2]
                        a21 = h1[:, kk, hi1:hi2]
                        a22 = h1[:, 24 + kk, hi1:hi2]
                        tt(S2A[1][:, kk, :], a11, a22, ADD)
                        tt(S2A[2][:, kk, :], a21, a22, ADD)
                        tt(S2A[5][:, kk, :], a11, a12, ADD)
                        tt(S2A[6][:, kk, :], a21, a11, SUB)
                        tt(S2A[7][:, kk, :], a12, a22, SUB)

                    # ---------------- fc1 ----------------
                    for q in range(MQ):
                        if q == 0:
                            def lhs1(k, mi):
                                return w1q0[k][:, mi * 128:(mi + 1) * 128]
                        else:
                            w1q = w1pool.tile([128, KT1, MPQ * 128],
                                              dt.float16, tag="w1")
                            nc.sync.dma_start(out=w1q[:], in_=w1t[q])

                            def lhs1(k, mi, w1q=w1q):
                                return w1q[:, k, mi * 128:(mi + 1) * 128]
                        for mi in range(MPQ):
                            m = q * MPQ + mi
                            psum = ps1.tile([128, BC], dt.float32, tag="ps1")
                            for k in range(KT1):
                                for n in range(NH):
                                    nc.tensor.matmul(
                                        psum[:, n * 512:(n + 1) * 512],
                                        lhs1(k, mi),
                                        xt_half[(k, n)],
                                        start=(k == 0),
                                        stop=(k == KT1 - 1),
                                    )
                            nc.scalar.sign(h1[:, m, :], psum[:, :],
                                           bias=b1_sb[:, m:m + 1])
                            if m >= 24:
                                s2_chunks(m - 24, 0)

                # ------------- fc2 (one-level Strassen) -------------
                with tc.tile_pool(name="s2b", bufs=1) as s2bpool, \
                     tc.tile_pool(name="w2pool", bufs=8) as w2pool, \
                     tc.tile_pool(name="c2pool", bufs=1) as cp2, \
                     tc.tile_pool(name="ps2", bufs=1, space="PSUM") as ps2:
                    dmae = [nc.sync, nc.gpsimd]
                    nd = [0]

                    # px=1 S-tiles: whole-tile builds on the (idle) DVE while
                    # the px=0 groups run on the PE
                    c1 = 256
                    A11b = h1[:, 0:24, c1:c1 + 256]
                    A12b = h1[:, 24:48, c1:c1 + 256]
                    A21b = h1[:, 0:24, 512 + c1:512 + c1 + 256]
                    A22b = h1[:, 24:48, 512 + c1:512 + c1 + 256]
                    S2B = {}
                    for i, a, b_, op in ((7, A12b, A22b, SUB),
                                         (5, A11b, A12b, ADD),
                                         (1, A11b, A22b, ADD),
                                         (2, A21b, A22b, ADD),
                                         (6, A21b, A11b, SUB)):
                        t = s2bpool.tile([128, 24, 256], dt.float8e4,
                                         tag=f"s2b{i}", name=f"s2b{i}")
                        tt(t[:], a, b_, op)
                        S2B[i] = t

                    # fc2 loops: f outer (each f's 7 weight tiles are
                    # loaded once and serve both px quarters), px inner
                    for f in range(NF):
                        wts = {}
                        for i in ORDER:
                            if f == 0 and i in w2pre:
                                wts[i] = w2pre[i]
                            else:
                                t = w2pool.tile([128, KBS, 2, 128],
                                                dt.float8e4, tag="w2")
                                eng = dmae[nd[0] % 2]
                                nd[0] += 1
                                eng.dma_start(out=t[:],
                                              in_=w2s[f, :, i - 1])
                                wts[i] = t
                        for px in range(2):
                            c0 = px * 256
                            S = S2A if px == 0 else S2B

                            def moving2(i, blk, c0=c0, S=S):
                                if i == 3:
                                    return h1[:, 2 * blk:2 * blk + 2,
                                              c0:c0 + 256]
                                if i == 4:
                                    return h1[:,
                                              24 + 2 * blk:24 + 2 * blk + 2,
                                              512 + c0:512 + c0 + 256]
                                return S[i][:, 2 * blk:2 * blk + 2, :]
                            # 7 quarter-size products packed
                            # pairwise into 2KB PSUM banks; pairs chosen by
                            # lifetime so bufs=1 tags never stall the PE
                            pA = ps2.tile([128, 512], dt.float32, tag="pA",
                                          name="pA")
                            pB = ps2.tile([128, 512], dt.float32, tag="pB",
                                          name="pB")
                            pC = ps2.tile([128, 512], dt.float32, tag="pC",
                                          name="pC")
                            pD = ps2.tile([128, 256], dt.float32, tag="pD",
                                          name="pD")
                            psm = {4: pA[:, 0:256], 7: pA[:, 256:512],
                                   5: pB[:, 0:256], 1: pB[:, 256:512],
                                   2: pC[:, 0:256], 3: pC[:, 256:512],
                                   6: pD[:, 0:256]}

                            def ctile(tag, bufs=1):
                                return cp2.tile([128, 256], dt.float32,
                                                tag=tag, name=tag, bufs=bufs)
                            for i in ORDER:
                                for blk in range(KBS):
                                    nc.tensor.matmul(
                                        psm[i], wts[i][:, blk],
                                        moving2(i, blk),
                                        start=(blk == 0),
                                        stop=(blk == KBS - 1),
                                        perf_mode=DR,
                                    )
                                if i == 4:
                                    m4s = ctile("m4s")
                                    nc.scalar.activation(m4s[:], psm[4],
                                                         AF.Identity)
                                elif i == 7:
                                    x1 = ctile("x1")
                                    tt(x1[:], m4s[:], psm[7], ADD)
                                elif i == 5:
                                    m5s = ctile("m5s")
                                    nc.scalar.activation(m5s[:], psm[5],
                                                         AF.Identity)
                                elif i == 1:
                                    x2 = ctile("x2")
                                    tt(x2[:], x1[:], psm[1], ADD)
                                    c11 = ctile("c11")
                                    tt(c11[:], x2[:], m5s[:], SUB)
                                    nc.scalar.sign(h2[:, f, c0:c0 + 256],
                                                   c11[:],
                                                   bias=b2_sb[:, f:f + 1])
                                elif i == 2:
                                    c21 = ctile("c21")
                                    tt(c21[:], m4s[:], psm[2], ADD)
                                    nc.scalar.sign(
                                        h2[:, f, 512 + c0:512 + c0 + 256],
                                        c21[:], bias=b2_sb[:, f:f + 1])
                                    m2s = ctile("m2s")
                                    nc.scalar.activation(m2s[:], psm[2],
                                                         AF.Identity)
                                elif i == 3:
                                    c12 = ctile("c12")
                                    tt(c12[:], m5s[:], psm[3], ADD)
                                    nc.scalar.sign(
                                        h2[:, 24 + f, c0:c0 + 256],
                                        c12[:],
                                        bias=b2_sb[:, 24 + f:25 + f])
                                    y2 = ctile("y2")
                                    nc.vector.scalar_tensor_tensor(
                                        y2[:], m2s[:], -1.0, psm[1],
                                        MULT, ADD)
                                    y3 = ctile("y3")
                                    tt(y3[:], y2[:], psm[3], ADD)
                                elif i == 6:
                                    c22 = ctile("c22")
                                    tt(c22[:], y3[:], psm[6], ADD)
                                    nc.scalar.sign(
                                        h2[:, 24 + f,
                                           512 + c0:512 + c0 + 256],
                                        c22[:],
                                        bias=b2_sb[:, 24 + f:25 + f])

            # ------------- fc3 (Strassen) + fused fc4 -------------
            with tc.tile_pool(name="lgp", bufs=1, space="PSUM") as lgp, \
                 tc.tile_pool(name="lgsbp", bufs=1) as lgsbp:
                lg_psum = lgp.tile([DOUT, BC], dt.float32)
                with tc.tile_pool(name="s3pool", bufs=1) as s3pool, \
                     tc.tile_pool(name="w3pool", bufs=8) as w3pool, \
                     tc.tile_pool(name="c3pool", bufs=1) as cp3, \
                     tc.tile_pool(name="h3pool", bufs=8) as h3pool, \
                     tc.tile_pool(name="ps3", bufs=1, space="PSUM") as ps3:
                    dmae3 = [nc.sync, nc.gpsimd]
                    nd3 = [0]
                    # all 10 S-tiles (both px quarters) up front
                    S3 = {}
                    for px in range(2):
                        c0 = px * 256
                        A11 = h2[:, 0:24, c0:c0 + 256]
                        A12 = h2[:, 24:48, c0:c0 + 256]
                        A21 = h2[:, 0:24, 512 + c0:512 + c0 + 256]
                        A22 = h2[:, 24:48, 512 + c0:512 + c0 + 256]
                        for i, a, b_, op in ((7, A12, A22, SUB),
                                             (5, A11, A12, ADD),
                                             (1, A11, A22, ADD),
                                             (2, A21, A22, ADD),
                                             (6, A21, A11, SUB)):
                            t = s3pool.tile([128, 24, 256], dt.float8e4,
                                            tag=f"s3_{i}_{px}",
                                            name=f"s3_{i}_{px}")
                            tt(t[:], a, b_, op)
                            S3[(i, px)] = t

                    h3_tiles = {}

                    def fc4_mms(m):
                        t_h3 = h3_tiles[m]
                        for n in range(NH):
                            nc.tensor.matmul(
                                lg_psum[:, n * 512:(n + 1) * 512],
                                w4_sb[:, m, :],
                                t_h3[:, n * 512:(n + 1) * 512],
                                start=(m == 0),
                                stop=(m == MT - 1),
                            )

                    def h3_store(m, cols, csb):
                        sl = h3_tiles[m][:, cols[0]:cols[1]]
                        nc.scalar.activation(sl, csb[:], AF.Identity,
                                             bias=b3_sb[:, m:m + 1])
                        nc.vector.tensor_scalar(sl, sl, 1.0, -1.0,
                                                mybir.AluOpType.min,
                                                mybir.AluOpType.max)

                    # fc3 loops: f outer (so h3/fc4 drain promptly), px inner
                    for f in range(NF):
                        for m in (f, 24 + f):
                            h3_tiles[m] = h3pool.tile([128, BC], dt.float16,
                                                      tag="h3", name="h3")
                        wts = {}
                        for i in ORDER:
                            if f == 0 and i in w3pre:
                                wts[i] = w3pre[i]
                            else:
                                t = w3pool.tile([128, KBS, 2, 128],
                                                dt.float8e4, tag="w3")
                                eng = dmae3[nd3[0] % 2]
                                nd3[0] += 1
                                eng.dma_start(out=t[:],
                                              in_=w3s[f, :, i - 1])
                                wts[i] = t
                        for px in range(2):
                            c0 = px * 256

                            def moving3(i, blk, c0=c0, px=px):
                                if i == 3:
                                    return h2[:, 2 * blk:2 * blk + 2,
                                              c0:c0 + 256]
                                if i == 4:
                                    return h2[:,
                                              24 + 2 * blk:24 + 2 * blk + 2,
                                              512 + c0:512 + c0 + 256]
                                return S3[(i, px)][:, 2 * blk:2 * blk + 2, :]
                            qA = ps3.tile([128, 512], dt.float32,
                                          tag="qA", name="qA")
                            qB = ps3.tile([128, 512], dt.float32,
                                          tag="qB", name="qB")
                            qC = ps3.tile([128, 512], dt.float32,
                                          tag="qC", name="qC")
                            qD = ps3.tile([128, 256], dt.float32,
                                          tag="qD", name="qD")
                            psm = {4: qA[:, 0:256], 7: qA[:, 256:512],
                                   5: qB[:, 0:256], 1: qB[:, 256:512],
                                   2: qC[:, 0:256], 3: qC[:, 256:512],
                                   6: qD[:, 0:256]}

                            def ctile3(tag, bufs=1):
                                return cp3.tile([128, 256], dt.float32,
                                                tag=tag, name=tag, bufs=bufs)
                            for i in ORDER:
                                for blk in range(KBS):
                                    nc.tensor.matmul(
                                        psm[i], wts[i][:, blk],
                                        moving3(i, blk),
                                        start=(blk == 0),
                                        stop=(blk == KBS - 1),
                                        perf_mode=DR,
                                    )
                                if i == 4:
                                    m4s = ctile3("f3m4s")
                                    nc.scalar.activation(m4s[:], psm[4],
                                                         AF.Identity)
                                elif i == 7:
                                    x1 = ctile3("f3x1")
                                    tt(x1[:], m4s[:], psm[7], ADD)
                                elif i == 5:
                                    m5s = ctile3("f3m5s")
                                    nc.scalar.activation(m5s[:], psm[5],
                                                         AF.Identity)
                                elif i == 1:
                                    x2 = ctile3("f3x2")
                                    tt(x2[:], x1[:], psm[1], ADD)
                                    c11 = ctile3("f3c11")
                                    tt(c11[:], x2[:], m5s[:], SUB)
                                    h3_store(f, (c0, c0 + 256), c11)
                                elif i == 2:
                                    c21 = ctile3("f3c21")
                                    tt(c21[:], m4s[:], psm[2], ADD)
                                    h3_store(f, (512 + c0, 512 + c0 + 256),
                                             c21)
                                    m2s = ctile3("f3m2s")
                                    nc.scalar.activation(m2s[:], psm[2],
                                                         AF.Identity)
                                elif i == 3:
                                    c12 = ctile3("f3c12")
                                    tt(c12[:], m5s[:], psm[3], ADD)
                                    h3_store(24 + f, (c0, c0 + 256), c12)
                                    y2 = ctile3("f3y2")
                                    nc.vector.scalar_tensor_tensor(
                                        y2[:], m2s[:], -1.0, psm[1],
                                        MULT, ADD)
                                    y3 = ctile3("f3y3")
                                    tt(y3[:], y2[:], psm[3], ADD)
                                elif i == 6:
                                    c22 = ctile3("f3c22")
                                    tt(c22[:], y3[:], psm[6], ADD)
                                    h3_store(24 + f,
                                             (512 + c0, 512 + c0 + 256), c22)
                        # fc4, pipelined one f behind
                        if f > 0:
                            fc4_mms(f - 1)
                            fc4_mms(24 + f - 1)
                    fc4_mms(NF - 1)
                    fc4_mms(24 + NF - 1)

                # ------------- bias + log_softmax (max-free) -------------
                # logits are bounded (|l| < 40), so exp() cannot overflow
                # fp32 and the rowmax subtraction is unnecessary:
                # out = l - ln(sum(exp(l))). Partition 32 of lg_sb holds the
                # per-column ln-sum so one PE transpose carries both.
                lg_sb = lgsbp.tile([33, BC], dt.float32)
                nc.scalar.activation(lg_sb[0:DOUT, :], lg_psum[:],
                                     AF.Identity, bias=b4_sb[:, 0:1])
                NJ = BC // 128
                with tc.tile_pool(name="tp", bufs=1, space="PSUM") as tpp, \
                     tc.tile_pool(name="sm", bufs=1) as smp:
                    ex_sb = smp.tile([DOUT, BC], dt.float16, tag="ex")
                    nc.scalar.activation(ex_sb[:], lg_psum[:], AF.Exp,
                                         bias=b4_sb[:, 0:1])
                    sums_ps = tpp.tile([1, BC], dt.float32, tag="sums")
                    for n in range(NH):
                        nc.tensor.matmul(
                            sums_ps[:, n * 512:(n + 1) * 512],
                            ones_sb[:, 0:1],
                            ex_sb[:, n * 512:(n + 1) * 512],
                        )
                    nc.scalar.activation(lg_sb[32:33, :], sums_ps[:], AF.Ln)
                    for j in range(NJ):
                        tp = tpp.tile([128, 33], dt.float32, tag=f"tp{j%4}")
                        nc.tensor.transpose(
                            tp[:], lg_sb[:, j * 128:(j + 1) * 128], ident[:])
                        res = smp.tile([128, DOUT], dt.float32,
                                       tag=f"res{j}")
                        nc.vector.tensor_scalar(res[:], tp[:, 0:DOUT],
                                                tp[:, 32:33], None,
                                                mybir.AluOpType.subtract)
                        nc.sync.dma_start(
                            out=out[j * 128:(j + 1) * 128, :], in_=res[:])

    nc.compile()
    return nc


def _pack_inputs(x, w1, b1, w2, b2, w3, b3, w4, b4):
    """Host-side packing into the device layouts. Shared tensors are packed
    once; only xt differs per core."""
    f32 = np.float32
    f16 = np.float16
    x = np.asarray(x, f32).reshape(B, DIN)

    # fc1 weights: sign(w1).T stacked twice (hi/lo terms share the weights),
    # padded to [1664, 6144], layout [q, p, k, m]
    s1 = np.sign(np.asarray(w1, f32))                       # [DH, DIN]
    s1t = np.zeros((K1P, DH), f16)
    s1t[:DIN] = s1.T
    s1t[DIN:2 * DIN] = s1.T
    w1t = np.ascontiguousarray(
        s1t.reshape(KT1, 128, MQ, MPQ * 128).transpose(2, 1, 0, 3))

    # fc2/fc3 weights: Strassen T-combos of sign(w).T, DoubleRow layout per
    # 128-wide output chunk: [fo, p, 7, blk, i2, f']
    def pack_strassen(w):
        sm = np.sign(np.asarray(w, f32)).T                  # [in, out] = B
        H = DH // 2
        B11 = sm[:H, :H]
        B12 = sm[:H, H:]
        B21 = sm[H:, :H]
        B22 = sm[H:, H:]
        Ts = [B11 + B22, B11, B12 - B22, B21 - B11, B22, B11 + B12,
              B21 + B22]

        def pack_t(t):   # [3072, 3072] -> [fo, p, blk, i2, f']
            r = t.reshape(KBS, 2, 128, NF, 128)
            return r.transpose(3, 2, 0, 1, 4)

        return np.ascontiguousarray(
            np.stack([pack_t(t) for t in Ts], axis=2)).astype(FP8)

    w2sp = pack_strassen(w2)
    w3sp = pack_strassen(w3)

    # fc4 weights: w4.T in fp16, layout [p, j, c]
    w4t = np.asarray(w4, f32).T.astype(f16)                 # [DH, DOUT]
    w4pk = np.ascontiguousarray(w4t.reshape(MT, 128, DOUT).transpose(1, 0, 2))

    def pack_b(b):
        return np.ascontiguousarray(np.asarray(b, f32).reshape(MT, 128).T)

    b1pk, b2pk, b3pk = pack_b(b1), pack_b(b2), pack_b(b3)
    b4pk = np.asarray(b4, f32).reshape(DOUT, 1)

    shared = {"w1t": w1t, "w2s": w2sp, "w3s": w3sp, "w4p": w4pk,
              "b1p": b1pk, "b2p": b2pk, "b3p": b3pk, "b4p": b4pk}

    # per-core x: fp16 hi/lo split stacked along contraction, layout [p, k, n]
    in_maps = []
    for c in range(CORES):
        xc = x[c * BC:(c + 1) * BC]                         # [BC, DIN]
        hi = xc.astype(f16)
        lo = (xc - hi.astype(f32)).astype(f16)
        arr = np.zeros((K1P, BC), f16)
        arr[:DIN] = hi.T
        arr[DIN:2 * DIN] = lo.T
        xtc = np.ascontiguousarray(arr.reshape(KT1, 128, BC).transpose(1, 0, 2))
        in_maps.append({"xt": xtc, **shared})
    return in_maps


_cached_nc = None


def kernel(x, w1, b1, w2, b2, w3, b3, w4, b4):
    global _cached_nc, last_exec_time_ns
    import os
    trace = bool(int(os.environ.get("KERNEL_TRACE", "0")))
    if _cached_nc is None:
        _cached_nc = _build_program()
    in_maps = _pack_inputs(x, w1, b1, w2, b2, w3, b3, w4, b4)
    res = run_bass_kernel_spmd(_cached_nc, in_maps, list(range(CORES)),
                               trace=trace)
    last_exec_time_ns = res.exec_time_ns
    return np.concatenate([res.results[c]["out"] for c in range(CORES)], axis=0)
